# revision 6
# baseline (speedup 1.0000x reference)
"""Local windowed self-attention layer (L=1024, B=8, D=512, E=1024, H=16, W=128)
on 8 TRN2 NeuronCores: data-parallel over batch (1 batch element per core).

Strategy:
  - Host prepacks transposed bf16 weights ([in,out] layout, the only layout the
    PE array can consume) and per-core transposed activations x^T.
  - On-chip everything is feature-major ([feat on partitions, seq on free]) so
    each matmul's PSUM output feeds the next matmul as the moving operand.
  - Windowed attention per (q-block 128, head): banded scores [128 x 384] with
    triangular window masks + key-padding mask added IN PSUM via extra matmuls,
    softmax via ACT exp (free accum_out denominators), attention-map head-sum
    via diag(1/16) identity-matmul PSUM accumulation, AV via PE-transposed
    probabilities against a seq-major V (computed directly by swapping matmul
    operand roles in the QKV projection).
  - LayerNorm over features (=partitions) via ones-vector matmul column sums;
    gamma folded into the output weight on host; per-position scale/shift
    applied at the final PSUM eviction.
"""

import numpy as np
import ml_dtypes
from contextlib import ExitStack

import concourse.bass as bass
import concourse.tile as tile
from concourse import bacc
from concourse import mybir
from concourse.bass_utils import run_bass_kernel_spmd

L, B, D, E, H = 1024, 8, 512, 1024, 16
HD = E // H          # 64
W = 128              # window size (hardcoded; matches reference setup)
EPS = 1e-5
NEG = -1e30

BF16 = mybir.dt.bfloat16
F32 = mybir.dt.float32
NPBF = ml_dtypes.bfloat16

QB = L // 128        # 8 query blocks
EC = E // 128        # 8 feature chunks of E
DC = D // 128        # 4 feature chunks of D

LAST_RESULTS = None  # test.py reads exec_time_ns / trace path from here
_CACHE = {}


def _emit(tc, t):
    nc = tc.nc

    # ---- constants in SBUF (once) ----
    with tc.tile_pool(name="consts", bufs=1) as cpool:
        ident = cpool.tile([128, 128], BF16, tag="ident", name="ident")
        nc.sync.dma_start(ident[:], t["ident"][:, :])
        tri_a = cpool.tile([128, 128], BF16, tag="tri_a", name="tri_a")
        nc.sync.dma_start(tri_a[:], t["tri_a"][:, :])
        tri_c = cpool.tile([128, 128], BF16, tag="tri_c", name="tri_c")
        nc.sync.dma_start(tri_c[:], t["tri_c"][:, :])
        diag16 = cpool.tile([128, 128], BF16, tag="diag16", name="diag16")
        nc.sync.dma_start(diag16[:], t["diag16"][:, :])
        ones1 = cpool.tile([1, 128], BF16, tag="ones1", name="ones1")
        nc.sync.dma_start(ones1[:], t["ones1"][:, :])
        ones_e = cpool.tile([128, 1], BF16, tag="ones_e", name="ones_e")
        nc.sync.dma_start(ones_e[:], t["ones_e"][:, :])
        one11 = cpool.tile([1, 1], F32, tag="one11", name="one11")
        nc.sync.dma_start(one11[:], t["one11"][:, :])
        seqmask = cpool.tile([1, L], BF16, tag="seqmask", name="seqmask")
        nc.sync.dma_start(seqmask[:], t["seqmask"][:, :])
        b_in_c = cpool.tile([128, EC], F32, tag="b_in_c", name="b_in_c")   # [128, 8] column per chunk
        nc.sync.dma_start(b_in_c[:], t["b_in_c"][:, :])
        u_bc = cpool.tile([128, D], F32, tag="u_bc", name="u_bc")
        nc.sync.dma_start(u_bc[:], t["u_bc"][:, :])
        w_bc = cpool.tile([128, D], F32, tag="w_bc", name="w_bc")
        nc.sync.dma_start(w_bc[:], t["w_bc"][:, :])

        # ---- long-lived activation buffers ----
        with tc.tile_pool(name="acts", bufs=1) as apool:
            ht = [apool.tile([128, L], BF16, tag=f"ht{i}", name=f"ht{i}") for i in range(EC)]
            qkt = [apool.tile([128, L], BF16, tag=f"qkt{i}", name=f"qkt{i}") for i in range(2 * EC)]
            v_sb = [apool.tile([128, E], BF16, tag=f"v{i}", name=f"v{i}") for i in range(QB)]
            avt = [apool.tile([128, L], BF16, tag=f"avt{i}", name=f"avt{i}") for i in range(EC)]
            rt = [apool.tile([128, L], BF16, tag=f"rt{i}", name=f"rt{i}") for i in range(EC)]

            # ================= stage B: hT = W_inT.T @ xT (+b_in) ==========
            with tc.tile_pool(name="stgb", bufs=2) as wpool, \
                 tc.tile_pool(name="psb", bufs=4, space="PSUM") as pp:
                xt = [wpool.tile([128, L], BF16, tag=f"xt{i}", name=f"xt{i}") for i in range(DC)]
                for i in range(DC):
                    nc.sync.dma_start(xt[i][:], t["xt"][i * 128:(i + 1) * 128, :])
                wint = [wpool.tile([128, E], BF16, tag=f"wint{i}", name=f"wint{i}") for i in range(DC)]
                for i in range(DC):
                    nc.sync.dma_start(wint[i][:], t["wint"][i * 128:(i + 1) * 128, :])
                for ec in range(EC):
                    for lc in range(2):
                        ps = pp.tile([128, 512], F32, tag="mm", name="mm")
                        for dc in range(DC):
                            nc.tensor.matmul(
                                ps[:],
                                wint[dc][:, ec * 128:(ec + 1) * 128],
                                xt[dc][:, lc * 512:(lc + 1) * 512],
                                start=(dc == 0), stop=(dc == DC - 1))
                        nc.any.tensor_scalar_add(
                            ht[ec][:, lc * 512:(lc + 1) * 512], ps[:],
                            b_in_c[:, ec:ec + 1])

            # ================= stage C: qT,kT feature-major; V seq-major ====
            with tc.tile_pool(name="stgc", bufs=1) as wpool, \
                 tc.tile_pool(name="psc", bufs=4, space="PSUM") as pp:
                wqkvt = [wpool.tile([128, 3 * E], BF16, tag=f"wq{i}", name=f"wq{i}") for i in range(EC)]
                for i in range(EC):
                    nc.sync.dma_start(wqkvt[i][:], t["wqkvt"][i * 128:(i + 1) * 128, :])
                # qT,kT: [2E, L] feature-major
                for fc in range(2 * EC):
                    for lc in range(2):
                        ps = pp.tile([128, 512], F32, tag="mm", name="mm")
                        for ec in range(EC):
                            nc.tensor.matmul(
                                ps[:],
                                wqkvt[ec][:, fc * 128:(fc + 1) * 128],
                                ht[ec][:, lc * 512:(lc + 1) * 512],
                                start=(ec == 0), stop=(ec == EC - 1))
                        nc.any.tensor_copy(qkt[fc][:, lc * 512:(lc + 1) * 512], ps[:])
                # V seq-major: v[l, f] -- swap roles: lhsT = hT chunk
                for lb in range(QB):
                    for vc in range(2):
                        ps = pp.tile([128, 512], F32, tag="mm", name="mm")
                        for ec in range(EC):
                            nc.tensor.matmul(
                                ps[:],
                                ht[ec][:, lb * 128:(lb + 1) * 128],
                                wqkvt[ec][:, 2 * E + vc * 512:2 * E + (vc + 1) * 512],
                                start=(ec == 0), stop=(ec == EC - 1))
                        nc.any.tensor_copy(v_sb[lb][:, vc * 512:(vc + 1) * 512], ps[:])

            # ================= stage D: windowed attention =================
            with tc.tile_pool(name="stgd", bufs=3) as dpool, \
                 tc.tile_pool(name="ps_s", bufs=2, space="PSUM") as pp_s, \
                 tc.tile_pool(name="ps_att", bufs=2, space="PSUM") as pp_att, \
                 tc.tile_pool(name="ps_pst", bufs=2, space="PSUM") as pp_pst, \
                 tc.tile_pool(name="ps_avt", bufs=2, space="PSUM") as pp_avt:
                for qb in range(QB):
                    k0 = max(0, (qb - 1) * 128)
                    k1 = min(L, (qb + 2) * 128)
                    kw = k1 - k0
                    kblocks = list(range(k0 // 128, k1 // 128))
                    att_ps = pp_att.tile([128, 384], F32, tag="att", name="att")
                    for h in range(H):
                        hc, off = divmod(h, 2)
                        off *= HD
                        qs = qkt[hc][off:off + HD, qb * 128:(qb + 1) * 128]
                        ks = qkt[EC + hc][off:off + HD, k0:k1]
                        # scores + masks accumulated in PSUM
                        s_ps = pp_s.tile([128, 384], F32, tag="s", name="s")
                        nc.tensor.matmul(s_ps[:, :kw], qs, ks, start=True, stop=False)
                        if qb > 0:
                            nc.tensor.matmul(s_ps[:, 0:128], tri_a[:], ident[:],
                                             start=False, stop=False)
                        if qb < QB - 1:
                            nc.tensor.matmul(s_ps[:, kw - 128:kw], tri_c[:], ident[:],
                                             start=False, stop=False)
                        nc.tensor.matmul(s_ps[:, :kw], ones1[:],
                                         seqmask[:, k0:k1], start=False, stop=True)
                        # softmax (no max subtraction needed; scores are tiny)
                        exps = dpool.tile([128, 384], BF16, tag="exps", name="exps")
                        denom = dpool.tile([128, 1], F32, tag="denom", name="denom")
                        nc.scalar.activation(exps[:, :kw], s_ps[:, :kw],
                                             mybir.ActivationFunctionType.Exp,
                                             scale=0.125, accum_out=denom[:])
                        recip = dpool.tile([128, 1], F32, tag="recip", name="recip")
                        nc.vector.tensor_scalar_add(denom[:], denom[:], 1e-30)
                        nc.vector.reciprocal(recip[:], denom[:])
                        ps_hat = dpool.tile([128, 384], BF16, tag="ps_hat", name="ps_hat")
                        nc.vector.tensor_scalar_mul(ps_hat[:, :kw], exps[:, :kw],
                                                    recip[:])
                        # att head-sum: att += (1/16) * ps_hat  (identity matmul)
                        nc.tensor.matmul(att_ps[:, :kw], diag16[:], ps_hat[:, :kw],
                                         start=(h == 0), stop=(h == H - 1))
                        # transpose ps_hat blocks -> [k, q]
                        pst_ps = pp_pst.tile([128, 384], BF16, tag="pst", name="pst")
                        for j in range(len(kblocks)):
                            nc.tensor.transpose(pst_ps[:, j * 128:(j + 1) * 128],
                                                ps_hat[:, j * 128:(j + 1) * 128],
                                                ident[:])
                        pst = dpool.tile([128, 384], BF16, tag="pst_sb", name="pst_sb")
                        nc.any.tensor_copy(pst[:, :kw], pst_ps[:, :kw])
                        # AV: avT[d, q] = sum_k v[k, d] * pshatT[k, q]
                        av_ps = pp_avt.tile([128, 128], F32, tag="avt", name="avt")
                        oav = av_ps[off:off + HD, :]
                        for j, kb in enumerate(kblocks):
                            nc.tensor.matmul(
                                oav, v_sb[kb][:, h * HD:(h + 1) * HD],
                                pst[:, j * 128:(j + 1) * 128],
                                start=(j == 0), stop=(j == len(kblocks) - 1))
                        nc.any.tensor_copy(
                            avt[hc][off:off + HD, qb * 128:(qb + 1) * 128], oav)
                    att_sb = dpool.tile([128, 384], F32, tag="att_sb", name="att_sb")
                    nc.any.tensor_copy(att_sb[:, :kw], att_ps[:, :kw])
                    nc.sync.dma_start(t["att"][qb * 128:(qb + 1) * 128, k0:k1],
                                      att_sb[:, :kw])

            # ======= stage E: attn out-proj (feature-major) + residual =====
            with tc.tile_pool(name="stge", bufs=1) as wpool, \
                 tc.tile_pool(name="pse", bufs=4, space="PSUM") as pp:
                watt = [wpool.tile([128, E], BF16, tag=f"watt{i}", name=f"watt{i}") for i in range(EC)]
                for i in range(EC):
                    nc.sync.dma_start(watt[i][:], t["watt"][i * 128:(i + 1) * 128, :])
                for ec in range(EC):
                    for lc in range(2):
                        ps = pp.tile([128, 512], F32, tag="mm", name="mm")
                        for fc in range(EC):
                            nc.tensor.matmul(
                                ps[:],
                                watt[fc][:, ec * 128:(ec + 1) * 128],
                                avt[fc][:, lc * 512:(lc + 1) * 512],
                                start=(fc == 0), stop=(fc == EC - 1))
                        # r = h + attn_out
                        nc.vector.tensor_add(rt[ec][:, lc * 512:(lc + 1) * 512],
                                             ps[:], ht[ec][:, lc * 512:(lc + 1) * 512])

            # ===== stage F: LN stats via ones-matmuls + final projection ====
            with tc.tile_pool(name="stgf", bufs=2) as fpool, \
                 tc.tile_pool(name="psf", bufs=1, space="PSUM") as pp, \
                 tc.tile_pool(name="psf2", bufs=2, space="PSUM") as pp2, \
                 tc.tile_pool(name="psy", bufs=3, space="PSUM") as ppy:
                wottg = [fpool.tile([128, D], BF16, tag=f"wog{i}", name=f"wog{i}") for i in range(EC)]
                for i in range(EC):
                    nc.sync.dma_start(wottg[i][:], t["wottg"][i * 128:(i + 1) * 128, :])
                a_row = fpool.tile([1, L], F32, tag="a_row", name="a_row")
                c_row = fpool.tile([1, L], F32, tag="c_row", name="c_row")
                for lc in range(2):
                    sl = slice(lc * 512, (lc + 1) * 512)
                    msum = pp.tile([1, 512], F32, tag="msum", name="msum")
                    for ec in range(EC):
                        nc.tensor.matmul(msum[:], ones_e[:], rt[ec][:, sl],
                                         start=(ec == 0), stop=(ec == EC - 1))
                    vsum = pp.tile([1, 512], F32, tag="vsum", name="vsum")
                    for ec in range(EC):
                        sq = fpool.tile([128, 512], BF16, tag="sq", name="sq")
                        nc.scalar.activation(sq[:], rt[ec][:, sl],
                                             mybir.ActivationFunctionType.Square)
                        nc.tensor.matmul(vsum[:], ones_e[:], sq[:],
                                         start=(ec == 0), stop=(ec == EC - 1))
                    mean = fpool.tile([1, 512], F32, tag="mean", name="mean")
                    nc.vector.tensor_scalar_mul(mean[:], msum[:], 1.0 / E)
                    m2 = fpool.tile([1, 512], F32, tag="m2", name="m2")
                    nc.vector.tensor_mul(m2[:], mean[:], mean[:])
                    var = fpool.tile([1, 512], F32, tag="var", name="var")
                    nc.vector.scalar_tensor_tensor(
                        var[:], vsum[:], 1.0 / E, m2[:],
                        op0=mybir.AluOpType.mult, op1=mybir.AluOpType.subtract)
                    nc.vector.tensor_scalar_add(var[:], var[:], EPS)
                    sd = fpool.tile([1, 512], F32, tag="sd", name="sd")
                    nc.scalar.activation(sd[:], var[:],
                                         mybir.ActivationFunctionType.Sqrt)
                    nc.vector.reciprocal(a_row[:, sl], sd[:])
                    # c = -mean * rstd
                    nc.vector.scalar_tensor_tensor(
                        c_row[:, sl], mean[:], -1.0, a_row[:, sl],
                        op0=mybir.AluOpType.mult, op1=mybir.AluOpType.mult)
                # transpose a/c rows into per-l-chunk columns
                ac = []
                for i in range(QB):
                    acp = pp2.tile([128, 2], F32, tag="acp", name="acp")
                    nc.tensor.transpose(acp[:, 0:1],
                                        a_row[0:1, i * 128:(i + 1) * 128], one11[:])
                    nc.tensor.transpose(acp[:, 1:2],
                                        c_row[0:1, i * 128:(i + 1) * 128], one11[:])
                    acs = fpool.tile([128, 2], F32, tag=f"acs{i}", name=f"acs{i}")
                    nc.any.tensor_copy(acs[:], acp[:])
                    ac.append(acs)
                # final: y[l,d] = a[l]*(rT . wottg)[l,d] + c[l]*u[d] + w[d]
                for lb in range(QB):
                    ps = ppy.tile([128, 512], F32, tag="ymm", name="ymm")
                    for ec in range(EC):
                        nc.tensor.matmul(ps[:],
                                         rt[ec][:, lb * 128:(lb + 1) * 128],
                                         wottg[ec][:],
                                         start=(ec == 0), stop=(ec == EC - 1))
                    t2 = fpool.tile([128, 512], F32, tag="t2", name="t2")
                    nc.vector.scalar_tensor_tensor(
                        t2[:], u_bc[:], ac[lb][:, 1:2], w_bc[:],
                        op0=mybir.AluOpType.mult, op1=mybir.AluOpType.add)
                    y_sb = fpool.tile([128, 512], F32, tag="y_sb", name="y_sb")
                    nc.vector.scalar_tensor_tensor(
                        y_sb[:], ps[:], ac[lb][:, 0:1], t2[:],
                        op0=mybir.AluOpType.mult, op1=mybir.AluOpType.add)
                    nc.sync.dma_start(t["y"][lb * 128:(lb + 1) * 128, :], y_sb[:])


def _build():
    nc = bacc.Bacc(None, target_bir_lowering=False, debug=False)
    t = {}
    mk = lambda n, s, d: nc.dram_tensor(n, s, d, kind="ExternalInput")
    t["xt"] = mk("xt", [D, L], BF16)
    t["seqmask"] = mk("seqmask", [1, L], BF16)
    t["wint"] = mk("wint", [D, E], BF16)
    t["wqkvt"] = mk("wqkvt", [E, 3 * E], BF16)
    t["watt"] = mk("watt", [E, E], BF16)
    t["wottg"] = mk("wottg", [E, D], BF16)
    t["b_in_c"] = mk("b_in_c", [128, EC], F32)
    t["u_bc"] = mk("u_bc", [128, D], F32)
    t["w_bc"] = mk("w_bc", [128, D], F32)
    t["ident"] = mk("ident", [128, 128], BF16)
    t["tri_a"] = mk("tri_a", [128, 128], BF16)
    t["tri_c"] = mk("tri_c", [128, 128], BF16)
    t["diag16"] = mk("diag16", [128, 128], BF16)
    t["ones1"] = mk("ones1", [1, 128], BF16)
    t["ones_e"] = mk("ones_e", [128, 1], BF16)
    t["one11"] = mk("one11", [1, 1], F32)
    t["y"] = nc.dram_tensor("y", [L, D], F32, kind="ExternalOutput")
    t["att"] = nc.dram_tensor("att", [L, L], F32, kind="ExternalOutput")
    with tile.TileContext(nc) as tc:
        _emit(tc, {k: (v[:] if hasattr(v, "__getitem__") else v)
                   for k, v in t.items()})
    nc.compile()
    return nc


def kernel(x, seq_lengths, W_in, b_in, in_proj_w, attn_out_w, ln_g, ln_b,
           W_out, b_out, window_size=128, **_kw):
    global LAST_RESULTS
    x = np.asarray(x, np.float32)
    seq_lengths = np.asarray(seq_lengths).astype(np.int64)
    W_in = np.asarray(W_in, np.float32)
    b_in = np.asarray(b_in, np.float32)
    in_proj_w = np.asarray(in_proj_w, np.float32)
    attn_out_w = np.asarray(attn_out_w, np.float32)
    ln_g = np.asarray(ln_g, np.float32)
    ln_b = np.asarray(ln_b, np.float32)
    W_out = np.asarray(W_out, np.float32)
    b_out = np.asarray(b_out, np.float32)

    if "nc" not in _CACHE:
        _CACHE["nc"] = _build()
    nc = _CACHE["nc"]

    bf = lambda a: np.ascontiguousarray(a).astype(NPBF)
    f32c = lambda a: np.ascontiguousarray(a).astype(np.float32)
    shared = {
        "wint": bf(W_in.T),
        "wqkvt": bf(in_proj_w.T),
        "watt": bf(attn_out_w.T),
        "wottg": bf((W_out * ln_g[None, :]).T),
        "b_in_c": f32c(b_in.reshape(EC, 128).T),
        "u_bc": f32c(np.tile(W_out @ ln_g, (128, 1))),
        "w_bc": f32c(np.tile(W_out @ ln_b + b_out, (128, 1))),
        "ident": bf(np.eye(128)),
        "tri_a": bf(np.triu(np.full((128, 128), NEG, np.float32), 1)),
        "tri_c": bf(np.tril(np.full((128, 128), NEG, np.float32), -1)),
        "diag16": bf(np.eye(128) / 16.0),
        "ones1": bf(np.ones((1, 128))),
        "ones_e": bf(np.ones((128, 1))),
        "one11": np.ones((1, 1), np.float32),
    }
    karr = np.arange(L)
    in_maps = []
    for b in range(B):
        m = dict(shared)
        m["xt"] = bf(x[:, b, :].T)
        m["seqmask"] = bf(np.where(karr < int(seq_lengths[b]), 0.0, NEG)
                          .reshape(1, L))
        in_maps.append(m)

    res = run_bass_kernel_spmd(nc, in_maps, core_ids=list(range(B)))
    LAST_RESULTS = res
    y = np.stack([r["y"] for r in res.results], axis=1)
    att = np.stack([r["att"] for r in res.results], axis=1)
    return y, att
